# revision 47
# baseline (speedup 1.0000x reference)
"""Bidirectional custom-LSTM (B=32, S=512, I=1024, H=512) on 8 TRN2 NeuronCores.

Sharding: 8 cores = 4 batch groups x 2 directions (data-parallel batch, one
direction per core). The variable-length sequence reversal for the backward
direction is done on the HOST (input pre-reversed per batch, output
un-reversed), so all 8 cores run an identical forward-scan program with no
indirect DMA and no index tables.

Per-core program:
  Phase A: x_proj = x @ Wi^T + b       (dense PE matmul; x arrives
           pre-transposed from host, result stays resident in SBUF;
           interleaved into the scan's PE idle windows)
  Scan (512 steps), state kept in a "strip" layout ([32g+b] partitions):
    G = sel_t^T @ x_proj[tile]         (K=128 time-row selection matmul)
      + h @ Wh^T                       (4 col-tiled strip matmuls x 4 K-rounds,
                                        i/j gate columns before o, separate
                                        PSUM tiles so o streams off-path)
    S = sigmoid(G)      # all 3 gates: j pre-scaled by 2 so tanh(j)=2*sig(2j)-1
    c' += S_i*(S_j-c')  # c' = (1+c)/2  (sigmoid-space cell state, fp16)
    h = tanh(2c'-1)*S_o
    h -> DRAM (plain sequential DMA on the SP engine, batched per 4 steps)
"""
import numpy as np

import concourse.bass as bass
import concourse.mybir as mybir
import concourse.tile as tile
from concourse.masks import make_identity
from concourse.vector_clock import ScopedClock

F32 = mybir.dt.float32
BF16 = mybir.dt.float16  # fp16: 1-pass PE, 10-bit mantissa
FP8 = mybir.dt.float8e4  # e4m3
I32 = mybir.dt.int32
AF = mybir.ActivationFunctionType
ALU = mybir.AluOpType
DR = mybir.MatmulPerfMode.DoubleRow

B, S, IN, H = 32, 512, 1024, 512
B_CORE = 8
G3 = 3 * H
NSTRIP = 4
SW = G3 // NSTRIP  # 384
NT = S // 16  # 32 phase-A tiles

# fp8e4m3 + DoubleRow (K=256 per matmul, 0.5 cycles/col) for the recurrent
# h @ Wh^T matmuls. DEAD END on this toolchain: walrus ISA check
# s3d3_mm_valid_dst_partition rejects DoubleRow matmuls whose PSUM
# destination starts at partition 32g (the strip layout's col-tiling), and
# abandoning the strip layout would double the ACT/DVE free sizes. Keep off.
FP8_REC = False

# ---------------------------------------------------------------------------
# Workaround: this toolchain's walrus accepts at most ONE sync-wait per
# instruction; Tile attaches several. Hoist extras onto NoOps just before.
# ---------------------------------------------------------------------------
_MAX_WAITS = 1
_orig_add = tile.TileContext._add_instruction
_SEM_COUNTS: dict = {}


_COMPUTE_INSTS = frozenset([
    "InstMatmult", "InstLdweights", "InstTensorTensor", "InstTensorScalar",
    "InstTensorScalarPtr", "InstActivation", "InstTensorCopy", "InstMemset",
    "InstIota", "InstStreamTranspose", "InstNoOp",
])


def _split_waits_add(self, inst):
    si = getattr(inst, "sync_info", None)
    if si is not None and si.on_wait and type(inst).__name__ in _COMPUTE_INSTS:
        # A compute instruction's wait on its OWN engine's completion
        # semaphore is satisfied by in-order execution (no engine here issues
        # DMAs on its own sem) — drop it instead of spending a NoOp slot.
        eng = str(inst.engine).split(".")[-1]
        kept = [w for w in si.on_wait
                if not (w.ant_name or "").startswith(eng + "_")]
        if len(kept) != len(si.on_wait):
            si.on_wait = kept
    if si is not None and si.on_wait and len(si.on_wait) > _MAX_WAITS:
        waits = list(si.on_wait)
        # Order so that waits already satisfied at emission time (buffer-reuse
        # WARs whose threshold is behind the producer's running count) go on
        # the early NoOps; the likely true blocker stays on the instruction
        # itself, so no NoOp decode serializes after the blocker clears.
        def slack(w):
            sem = getattr(w, "semaphore", None)
            key = getattr(w, "ant_name", None) or str(sem)
            return _SEM_COUNTS.get(key, 0) - (w.wait_value or 0)
        waits.sort(key=slack)  # least-satisfied first
        si.on_wait = waits[-_MAX_WAITS:]
        head = waits[:-_MAX_WAITS]
        for i in range(0, len(head), _MAX_WAITS):
            nop = mybir.InstNoOp(
                name=self.nc.get_next_instruction_name(),
                engine=inst.engine, ins=[], outs=[],
                sync_info=mybir.SyncInfo(on_wait=head[i:i + _MAX_WAITS],
                                         on_update=[]),
            )
            _orig_add(self, nop)
    if si is not None and si.on_update:
        for u in si.on_update:
            key = getattr(u, "ant_name", None) or str(getattr(u, "semaphore", None))
            _SEM_COUNTS[key] = _SEM_COUNTS.get(key, 0) + 1
    _orig_add(self, inst)


def _drain_and_barrier_split(self, tick_clock, wait_clock):
    nc = self.nc
    probe = nc.sync.nop(nofuse=True, hint="tile_exit_waits")
    wait_clock.add_sem_waits(probe.ins, ScopedClock({None: tick_clock.global_clock}))
    si = probe.ins.sync_info
    waits = list(si.on_wait) if si and si.on_wait else []
    if len(waits) > _MAX_WAITS:
        si.on_wait = waits[:_MAX_WAITS]
        rest = waits[_MAX_WAITS:]
        while rest:
            chunk, rest = rest[:_MAX_WAITS], rest[_MAX_WAITS:]
            extra = nc.sync.nop(nofuse=True, hint="tile_exit_waits")
            extra.ins.sync_info = mybir.SyncInfo(on_wait=chunk, on_update=[])
    nc.sync.drain()
    nc.all_engine_barrier()
    assert self.sems is not None
    popped = nc._tile_sem_poison_stack.pop()
    assert popped is self._sem_poison
    nc.clear_and_free_semaphores(list(self.sems.allocated().values()))
    nc.all_engine_barrier()


tile.TileContext._add_instruction = _split_waits_add
tile.TileContext._drain_and_barrier = _drain_and_barrier_split


# ---------------------------------------------------------------------------
def build_nc():
    nc = bass.Bass(detect_race_conditions=False)
    # x pre-transposed on host: [m, feat, kchunk, row] with row = 8*tl + b
    xT_d = nc.dram_tensor("xT", [NT, 128, 8, 128], BF16, kind="ExternalInput")
    wiT_d = nc.dram_tensor("wiT", [IN, G3], BF16, kind="ExternalInput")
    if FP8_REC:
        whT_d = nc.dram_tensor("whT", [128, 4, G3], FP8, kind="ExternalInput")
    else:
        whT_d = nc.dram_tensor("whT", [H, G3], BF16, kind="ExternalInput")
    biasb_d = nc.dram_tensor("biasb", [128, G3], BF16, kind="ExternalInput")
    sel_d = nc.dram_tensor("sel", [128, 16 * 32], BF16, kind="ExternalInput")
    if FP8_REC:
        hT0_d = nc.dram_tensor("hT0", [128, 4, 32], FP8, kind="ExternalInput")
    else:
        hT0_d = nc.dram_tensor("hT0", [128, 128], BF16, kind="ExternalInput")
    cp0_d = nc.dram_tensor("cp0", [128, 128], BF16, kind="ExternalInput")
    # out[g, b, t, c] = h[b, t, 128*g + c]  (only b < 8 is real data)
    out_d = nc.dram_tensor("out", [4, 32, S, 128], BF16, kind="ExternalOutput")

    with tile.TileContext(nc) as tc:
        from contextlib import ExitStack
        with ExitStack() as ctx:
            sbc = ctx.enter_context(tc.tile_pool(name="sbc", bufs=1))
            sb = ctx.enter_context(tc.tile_pool(name="sb", bufs=2))
            sbh = ctx.enter_context(tc.tile_pool(name="sbh", bufs=2))

            # -------- constants / weights --------
            wiT = sbc.tile([128, 8 * G3], BF16, tag="wiT")
            for k in range(8):
                nc.sync.dma_start(out=wiT[:, k * G3:(k + 1) * G3],
                                  in_=wiT_d[128 * k:128 * (k + 1), :])
            if FP8_REC:
                whT = sbc.tile([128, 4, G3], FP8, tag="whT")
                nc.sync.dma_start(out=whT[:], in_=whT_d[:])
            else:
                whT = sbc.tile([128, 4 * G3], BF16, tag="whT")
                for k in range(4):
                    nc.sync.dma_start(out=whT[:, k * G3:(k + 1) * G3],
                                      in_=whT_d[128 * k:128 * (k + 1), :])
            biasb = sbc.tile([128, G3], BF16, tag="biasb")
            nc.sync.dma_start(out=biasb[:], in_=biasb_d[:])
            sel = sbc.tile([128, 16 * 32], BF16, tag="sel")
            nc.sync.dma_start(out=sel[:], in_=sel_d[:])
            neg1 = sbc.tile([128, 1], F32, tag="neg1")
            nc.vector.memset(neg1[:], -1.0)
            ident = sbc.tile([128, 128], F32, tag="ident")
            make_identity(nc, ident[:])
            ident_bf = sbc.tile([128, 128], BF16, tag="ident_bf")
            nc.vector.tensor_copy(out=ident_bf[:], in_=ident[:])

            # recurrent state: hT[:, 32k + b] = h[b, 128k + p]
            if FP8_REC:
                hT = sbc.tile([128, 4, 32], FP8, tag="hT")
            else:
                hT = sbc.tile([128, 128], BF16, tag="hT")
            nc.sync.dma_start(out=hT[:], in_=hT0_d[:])
            cp = sbc.tile([128, 128], BF16, tag="cp")
            nc.sync.dma_start(out=cp[:], in_=cp0_d[:])

            # x_proj, SBUF-resident for the whole scan: [128, NT*G3] bf16
            xp = sbc.tile([128, NT * G3], BF16, tag="xp")

            # -------- Phase A: xp = x @ Wi^T + bias (rows: 8*tl + b) ------
            # Interleaved with the scan: tile m is emitted at the boundary of
            # scan window m-2, filling PE idle while the recurrence's serial
            # chain runs. One [128,512] PSUM block at a time (bufs=2).
            sbA = ctx.enter_context(tc.tile_pool(name="sbA", bufs=2))
            psA = ctx.enter_context(tc.tile_pool(name="psA", bufs=2,
                                                 space="PSUM"))

            def emit_phase_a(m):
                xT = sbA.tile([128, 1024], BF16, tag="xT", name=f"xT_{m}")
                nc.sync.dma_start(out=xT[:], in_=xT_d[m])
                for n in range(3):
                    pps = psA.tile([128, 512], F32, tag="psA", name=f"psA_{m}_{n}")
                    for k in range(8):
                        nc.tensor.matmul(
                            out=pps[:],
                            lhsT=xT[:, k * 128:(k + 1) * 128],
                            rhs=wiT[:, k * G3 + 512 * n: k * G3 + 512 * (n + 1)],
                            start=(k == 0), stop=(k == 7))
                    nc.vector.tensor_tensor(
                        out=xp[:, G3 * m + 512 * n: G3 * m + 512 * (n + 1)],
                        in0=pps[:], in1=biasb[:, 512 * n:512 * (n + 1)],
                        op=ALU.add)

            emit_phase_a(0)
            emit_phase_a(1)

            # -------- scan --------
            ps = ctx.enter_context(tc.tile_pool(name="ps", bufs=2, space="PSUM"))
            pso = ctx.enter_context(tc.tile_pool(name="pso", bufs=2, space="PSUM"))
            ps2 = ctx.enter_context(tc.tile_pool(name="ps2", bufs=2, space="PSUM"))

            def emit_sel_mms(t):
                # x_proj selection matmuls for step t: no dependency on h, so
                # they are issued early to fill the PE idle window. The o-gate
                # columns go to a separate PSUM tile: PSUM dependency tracking
                # is bank-coarse, and sharing a bank would serialize the
                # o-phase matmuls behind sig_ij's read.
                m, tl = t >> 4, t & 15
                gps = ps.tile([128, 256], F32, tag="gps", name=f"gps_{t}")
                gpo = pso.tile([128, 128], F32, tag="gpo", name=f"gpo_{t}")
                for g in range(NSTRIP):
                    nc.tensor.matmul(
                        out=gps[32 * g:32 * (g + 1), :],
                        lhsT=sel[:, 32 * tl:32 * tl + 32],
                        rhs=xp[:, G3 * m + SW * g: G3 * m + SW * g + 256],
                        start=True, stop=False, tile_position=(0, 32 * g),
                        skip_group_check=True)
                    nc.tensor.matmul(
                        out=gpo[32 * g:32 * (g + 1), :],
                        lhsT=sel[:, 32 * tl:32 * tl + 32],
                        rhs=xp[:, G3 * m + SW * g + 256: G3 * m + SW * (g + 1)],
                        start=True, stop=False, tile_position=(0, 32 * g),
                        skip_group_check=True)
                return gps, gpo

            gps, gpo = emit_sel_mms(0)
            for t in range(S):
                if t % 16 == 0 and (t >> 4) + 2 < NT:
                    emit_phase_a((t >> 4) + 2)
                # recurrent matmuls, round-major so each round only needs its
                # own chunk(s) of hT (pipelines with the chunked copies below)
                def rec_mms(out_t, c0, c1):
                    for k in range(4):
                        for g in range(NSTRIP):
                            nc.tensor.matmul(
                                out=out_t[32 * g:32 * (g + 1), 0:c1 - c0],
                                lhsT=hT[:, 32 * k:32 * (k + 1)],
                                rhs=whT[:, k * G3 + SW * g + c0:
                                         k * G3 + SW * g + c1],
                                start=False, stop=(k == 3),
                                tile_position=(0, 32 * g),
                                skip_group_check=True)
                Sg = sb.tile([128, SW], BF16, tag="Sg", name=f"Sg_{t}")
                rec_mms(gps, 0, 256)   # i, j gate columns: the critical input
                nc.scalar.activation(Sg[:, 0:256], gps[:], AF.Sigmoid)
                rec_mms(gpo, 256, SW)  # o gate columns: consumed later by h
                nc.scalar.activation(Sg[:, 256:SW], gpo[:], AF.Sigmoid)
                d = sb.tile([128, 128], BF16, tag="d", name=f"d_{t}")
                nc.vector.tensor_tensor(out=d[:], in0=Sg[:, 128:256], in1=cp[:],
                                        op=ALU.subtract)
                e = sb.tile([128, 128], BF16, tag="e", name=f"e_{t}")
                nc.vector.tensor_tensor(out=e[:], in0=Sg[:, 0:128], in1=d[:],
                                        op=ALU.mult)
                nc.vector.tensor_tensor(out=cp[:], in0=cp[:], in1=e[:], op=ALU.add)
                tc_t = sb.tile([128, 128], BF16, tag="tc", name=f"tc_{t}")
                nc.scalar.activation(tc_t[:], cp[:], AF.Tanh, bias=neg1[:, 0:1],
                                     scale=2.0)
                if t % 4 == 0:
                    hst = sbh.tile([128, 512], BF16, tag="hst", name=f"hst_{t}")
                h = hst[:, 128 * (t % 4):128 * (t % 4 + 1)]
                nc.vector.tensor_tensor(out=h, in0=tc_t[:], in1=Sg[:, 256:SW],
                                        op=ALU.mult)
                if t % 4 == 3:
                    nc.sync.dma_start(out=out_d[:, :, t - 3:t + 1, :], in_=hst[:])
                if t < S - 1:
                    gps, gpo = emit_sel_mms(t + 1)
                    # chunked transpose: h partition-group -> hT column chunk;
                    # round k of the next step starts after copy k
                    hp = ps2.tile([128, 128], BF16, tag="hp", name=f"hp_{t}")
                    for j in range(2):
                        nc.tensor.transpose(
                            out=hp[:, 64 * j:64 * (j + 1)],
                            in_=hst[64 * j:64 * (j + 1),
                                    128 * (t % 4):128 * (t % 4 + 1)],
                            identity=ident_bf[64 * j:64 * (j + 1),
                                              64 * j:64 * (j + 1)],
                            tile_position=(64 * j, 0))
                        nc.vector.tensor_copy(
                            out=hT[:, 64 * j:64 * (j + 1)],
                            in_=hp[:, 64 * j:64 * (j + 1)])
    return nc


def prep_weights(Wi, bi, Wh, bh, h0, c0):
    perm = []
    for g in range(4):
        perm += list(range(128 * g, 128 * (g + 1)))
        perm += list(range(H + 128 * g, H + 128 * (g + 1)))
        perm += list(range(2 * H + 128 * g, 2 * H + 128 * (g + 1)))
    perm = np.array(perm)
    scale = np.ones(G3, np.float32)
    for g in range(4):
        scale[384 * g + 128:384 * g + 256] = 2.0  # j gates doubled
    Wi_p = (np.asarray(Wi)[perm] * scale[:, None]).astype(np.float32)
    Wh_p = (np.asarray(Wh)[perm] * scale[:, None]).astype(np.float32)
    bias = ((np.asarray(bi) + np.asarray(bh))[perm] * scale).astype(np.float32)
    h0f = np.asarray(h0).reshape(-1).astype(np.float32)
    c0pf = ((1.0 + np.asarray(c0).reshape(-1)) / 2.0).astype(np.float32)
    hT0 = np.zeros((128, 128), np.float32)
    cp0 = np.zeros((128, 128), np.float32)
    for k in range(4):
        hT0[:, 32 * k:32 * k + 8] = h0f[128 * k:128 * (k + 1)][:, None]
        cp0[32 * k:32 * k + 8, :] = c0pf[128 * k:128 * (k + 1)][None, :]
    bf16 = np.float16
    if FP8_REC:
        import ml_dtypes
        # whT[p, k, col] = Wh_p.T[128k + p, col]
        whT = np.ascontiguousarray(
            Wh_p.T.reshape(4, 128, G3).transpose(1, 0, 2)
        ).astype(ml_dtypes.float8_e4m3)
        hT0_o = hT0.reshape(128, 4, 32).astype(ml_dtypes.float8_e4m3)
    else:
        whT = np.ascontiguousarray(Wh_p.T).astype(bf16)
        hT0_o = hT0.astype(bf16)
    return {
        "wiT": np.ascontiguousarray(Wi_p.T).astype(bf16),
        "whT": whT,
        "biasb": np.ascontiguousarray(
            np.broadcast_to(bias, (128, G3))).astype(bf16),
    }, hT0_o, cp0.astype(bf16)


def _sel_matrix():
    sel = np.zeros((128, 16 * 32), np.float16)
    for tl in range(16):
        for b in range(B_CORE):
            sel[8 * tl + b, 32 * tl + b] = 1.0
    return sel


def _to_xT(x_core):
    """[8, 512, 1024] -> [NT, 128, 8, 128]: [m, f, k, 8*tl+b]."""
    a = x_core.reshape(B_CORE, NT, 16, 8, 128)  # b, m, tl, k, f
    return np.ascontiguousarray(a.transpose(1, 4, 3, 2, 0)).reshape(NT, 128, 8, 128)


_NC_CACHE = {}


def _get_nc():
    if "nc" not in _NC_CACHE:
        _NC_CACHE["nc"] = build_nc()
    return _NC_CACHE["nc"]


def make_in_maps(input, sent_len, fwd_h0, fwd_c0, fwd_Wi, fwd_bi, fwd_Wh, fwd_bh,
                 bwd_h0, bwd_c0, bwd_Wi, bwd_bi, bwd_Wh, bwd_bh, **_unused):
    x = np.asarray(input, np.float32).astype(np.float16)
    sent = np.asarray(sent_len).astype(np.int64)
    # per-batch reversal index: first sent_len[b] steps reversed, rest kept
    tt = np.arange(S)[None, :]
    L = sent[:, None]
    rev_idx = np.where(tt < L, L - 1 - tt, tt)  # [B, S]
    fwd_w, fwd_hT0, fwd_cp0 = prep_weights(fwd_Wi, fwd_bi, fwd_Wh, fwd_bh,
                                           fwd_h0, fwd_c0)
    bwd_w, bwd_hT0, bwd_cp0 = prep_weights(bwd_Wi, bwd_bi, bwd_Wh, bwd_bh,
                                           bwd_h0, bwd_c0)
    sel = _sel_matrix()
    in_maps = []
    for c in range(8):
        bs = (c % 4) * B_CORE
        xc = x[bs:bs + B_CORE]
        if c >= 4:
            xc = np.take_along_axis(xc, rev_idx[bs:bs + B_CORE, :, None], axis=1)
        w, hT0, cp0 = (fwd_w, fwd_hT0, fwd_cp0) if c < 4 else \
                      (bwd_w, bwd_hT0, bwd_cp0)
        m = dict(w)
        m["xT"] = _to_xT(xc)
        m["sel"] = sel
        m["hT0"] = hT0
        m["cp0"] = cp0
        in_maps.append(m)
    return in_maps, rev_idx


def kernel(**inputs):
    from concourse.bass_utils import run_bass_kernel_spmd

    in_maps, rev_idx = make_in_maps(**inputs)
    nc = _get_nc()
    res = run_bass_kernel_spmd(nc, in_maps, list(range(8)))
    out = np.zeros((B, S, 2 * H), np.float32)
    for c in range(8):
        bs = (c % 4) * B_CORE
        o = res.results[c]["out"]  # [4, 32, S, 128]
        o = o[:, :B_CORE].transpose(1, 2, 0, 3).reshape(B_CORE, S, H)
        if c < 4:
            out[bs:bs + B_CORE, :, :H] = o
        else:
            idx = rev_idx[bs:bs + B_CORE]
            out[bs:bs + B_CORE, :, H:] = np.take_along_axis(
                o, idx[:, :, None], axis=1)
    return out


# revision 58
# speedup vs baseline: 1.0015x; 1.0015x over previous
"""Bidirectional custom-LSTM (B=32, S=512, I=1024, H=512) on 8 TRN2 NeuronCores.

Sharding: 8 cores = 4 batch groups x 2 directions (data-parallel batch, one
direction per core). The variable-length sequence reversal for the backward
direction is done on the HOST (input pre-reversed per batch, output
un-reversed), so all 8 cores run an identical forward-scan program with no
indirect DMA and no index tables.

Per-core program:
  Phase A: x_proj = x @ Wi^T + b       (dense PE matmul; x arrives
           pre-transposed from host, result stays resident in SBUF;
           interleaved into the scan's PE idle windows)
  Scan (512 steps), state kept in a "strip" layout ([32g+b] partitions):
    G = sel_t^T @ x_proj[tile]         (K=128 time-row selection matmul)
      + h @ Wh^T                       (4 col-tiled strip matmuls x 4 K-rounds,
                                        i/j gate columns before o, separate
                                        PSUM tiles so o streams off-path)
    S = sigmoid(G)      # all 3 gates: j pre-scaled by 2 so tanh(j)=2*sig(2j)-1
    c' += S_i*(S_j-c')  # c' = (1+c)/2  (sigmoid-space cell state, fp16)
    h = tanh(2c'-1)*S_o
    h -> DRAM (plain sequential DMA on the SP engine, batched per 4 steps)
"""
import numpy as np

import concourse.bass as bass
import concourse.mybir as mybir
import concourse.tile as tile
from concourse.masks import make_identity
from concourse.vector_clock import ScopedClock

F32 = mybir.dt.float32
BF16 = mybir.dt.float16  # fp16: 1-pass PE, 10-bit mantissa
FP8 = mybir.dt.float8e4  # e4m3
I32 = mybir.dt.int32
AF = mybir.ActivationFunctionType
ALU = mybir.AluOpType
DR = mybir.MatmulPerfMode.DoubleRow

B, S, IN, H = 32, 512, 1024, 512
B_CORE = 8
G3 = 3 * H
NSTRIP = 4
SW = G3 // NSTRIP  # 384
NT = S // 16  # 32 phase-A tiles

# fp8e4m3 + DoubleRow (K=256 per matmul, 0.5 cycles/col) for the recurrent
# h @ Wh^T matmuls. DEAD END on this toolchain: walrus ISA check
# s3d3_mm_valid_dst_partition rejects DoubleRow matmuls whose PSUM
# destination starts at partition 32g (the strip layout's col-tiling), and
# abandoning the strip layout would double the ACT/DVE free sizes. Keep off.
FP8_REC = False

# ---------------------------------------------------------------------------
# Workaround: this toolchain's walrus accepts at most ONE sync-wait per
# instruction; Tile attaches several. Hoist extras onto NoOps just before.
# ---------------------------------------------------------------------------
_MAX_WAITS = 1
_orig_add = tile.TileContext._add_instruction
_SEM_COUNTS: dict = {}


_COMPUTE_INSTS = frozenset([
    "InstMatmult", "InstLdweights", "InstTensorTensor", "InstTensorScalar",
    "InstTensorScalarPtr", "InstActivation", "InstTensorCopy", "InstMemset",
    "InstIota", "InstStreamTranspose", "InstNoOp",
])


def _split_waits_add(self, inst):
    si = getattr(inst, "sync_info", None)
    if si is not None and si.on_wait and type(inst).__name__ in _COMPUTE_INSTS:
        # A compute instruction's wait on its OWN engine's completion
        # semaphore is satisfied by in-order execution (no engine here issues
        # DMAs on its own sem) — drop it instead of spending a NoOp slot.
        eng = str(inst.engine).split(".")[-1]
        kept = [w for w in si.on_wait
                if not (w.ant_name or "").startswith(eng + "_")]
        if len(kept) != len(si.on_wait):
            si.on_wait = kept
    if si is not None and si.on_wait and len(si.on_wait) > _MAX_WAITS:
        waits = list(si.on_wait)
        # Order so that waits already satisfied at emission time (buffer-reuse
        # WARs whose threshold is behind the producer's running count) go on
        # the early NoOps; the likely true blocker stays on the instruction
        # itself, so no NoOp decode serializes after the blocker clears.
        def slack(w):
            sem = getattr(w, "semaphore", None)
            key = getattr(w, "ant_name", None) or str(sem)
            return _SEM_COUNTS.get(key, 0) - (w.wait_value or 0)
        waits.sort(key=slack)  # least-satisfied first
        si.on_wait = waits[-_MAX_WAITS:]
        head = waits[:-_MAX_WAITS]
        for i in range(0, len(head), _MAX_WAITS):
            nop = mybir.InstNoOp(
                name=self.nc.get_next_instruction_name(),
                engine=inst.engine, ins=[], outs=[],
                sync_info=mybir.SyncInfo(on_wait=head[i:i + _MAX_WAITS],
                                         on_update=[]),
            )
            _orig_add(self, nop)
    if si is not None and si.on_update:
        for u in si.on_update:
            key = getattr(u, "ant_name", None) or str(getattr(u, "semaphore", None))
            _SEM_COUNTS[key] = _SEM_COUNTS.get(key, 0) + 1
    _orig_add(self, inst)


def _drain_and_barrier_split(self, tick_clock, wait_clock):
    nc = self.nc
    probe = nc.sync.nop(nofuse=True, hint="tile_exit_waits")
    wait_clock.add_sem_waits(probe.ins, ScopedClock({None: tick_clock.global_clock}))
    si = probe.ins.sync_info
    waits = list(si.on_wait) if si and si.on_wait else []
    if len(waits) > _MAX_WAITS:
        si.on_wait = waits[:_MAX_WAITS]
        rest = waits[_MAX_WAITS:]
        while rest:
            chunk, rest = rest[:_MAX_WAITS], rest[_MAX_WAITS:]
            extra = nc.sync.nop(nofuse=True, hint="tile_exit_waits")
            extra.ins.sync_info = mybir.SyncInfo(on_wait=chunk, on_update=[])
    nc.sync.drain()
    nc.all_engine_barrier()
    assert self.sems is not None
    popped = nc._tile_sem_poison_stack.pop()
    assert popped is self._sem_poison
    nc.clear_and_free_semaphores(list(self.sems.allocated().values()))
    nc.all_engine_barrier()


tile.TileContext._add_instruction = _split_waits_add
tile.TileContext._drain_and_barrier = _drain_and_barrier_split


# ---------------------------------------------------------------------------
def build_nc():
    nc = bass.Bass(detect_race_conditions=False)
    # x pre-transposed on host: [m, feat, kchunk, row] with row = 8*tl + b
    xT_d = nc.dram_tensor("xT", [NT, 128, 8, 128], BF16, kind="ExternalInput")
    wiT_d = nc.dram_tensor("wiT", [IN, G3], BF16, kind="ExternalInput")
    if FP8_REC:
        whT_d = nc.dram_tensor("whT", [128, 4, G3], FP8, kind="ExternalInput")
    else:
        whT_d = nc.dram_tensor("whT", [H, G3], BF16, kind="ExternalInput")
    biasb_d = nc.dram_tensor("biasb", [128, G3], BF16, kind="ExternalInput")
    sel_d = nc.dram_tensor("sel", [128, 16 * 32], BF16, kind="ExternalInput")
    if FP8_REC:
        hT0_d = nc.dram_tensor("hT0", [128, 4, 32], FP8, kind="ExternalInput")
    else:
        hT0_d = nc.dram_tensor("hT0", [128, 128], BF16, kind="ExternalInput")
    cp0_d = nc.dram_tensor("cp0", [128, 128], BF16, kind="ExternalInput")
    # out[g, b, t, c] = h[b, t, 128*g + c]  (only b < 8 is real data)
    out_d = nc.dram_tensor("out", [4, 32, S, 128], BF16, kind="ExternalOutput")

    with tile.TileContext(nc) as tc:
        from contextlib import ExitStack
        with ExitStack() as ctx:
            sbc = ctx.enter_context(tc.tile_pool(name="sbc", bufs=1))
            sb = ctx.enter_context(tc.tile_pool(name="sb", bufs=2))
            sbh = ctx.enter_context(tc.tile_pool(name="sbh", bufs=2))

            # -------- constants / weights --------
            wiT = sbc.tile([128, 8 * G3], BF16, tag="wiT")
            for k in range(8):
                nc.sync.dma_start(out=wiT[:, k * G3:(k + 1) * G3],
                                  in_=wiT_d[128 * k:128 * (k + 1), :])
            if FP8_REC:
                whT = sbc.tile([128, 4, G3], FP8, tag="whT")
                nc.sync.dma_start(out=whT[:], in_=whT_d[:])
            else:
                whT = sbc.tile([128, 4 * G3], BF16, tag="whT")
                for k in range(4):
                    nc.sync.dma_start(out=whT[:, k * G3:(k + 1) * G3],
                                      in_=whT_d[128 * k:128 * (k + 1), :])
            biasb = sbc.tile([128, G3], BF16, tag="biasb")
            nc.sync.dma_start(out=biasb[:], in_=biasb_d[:])
            sel = sbc.tile([128, 16 * 32], BF16, tag="sel")
            nc.sync.dma_start(out=sel[:], in_=sel_d[:])
            neg1 = sbc.tile([128, 1], F32, tag="neg1")
            nc.vector.memset(neg1[:], -1.0)
            ident = sbc.tile([128, 128], F32, tag="ident")
            make_identity(nc, ident[:])
            ident_bf = sbc.tile([128, 128], BF16, tag="ident_bf")
            nc.vector.tensor_copy(out=ident_bf[:], in_=ident[:])

            # recurrent state: hT[:, 32k + b] = h[b, 128k + p]
            if FP8_REC:
                hT = sbc.tile([128, 4, 32], FP8, tag="hT")
            else:
                hT = sbc.tile([128, 128], BF16, tag="hT")
            nc.sync.dma_start(out=hT[:], in_=hT0_d[:])
            cp = sbc.tile([128, 128], BF16, tag="cp")
            nc.sync.dma_start(out=cp[:], in_=cp0_d[:])

            # x_proj, SBUF-resident for the whole scan: [128, NT*G3] bf16
            xp = sbc.tile([128, NT * G3], BF16, tag="xp")

            # -------- Phase A: xp = x @ Wi^T + bias (rows: 8*tl + b) ------
            # Interleaved with the scan: tile m is emitted at the boundary of
            # scan window m-2, filling PE idle while the recurrence's serial
            # chain runs. One [128,512] PSUM block at a time (bufs=2).
            sbA = ctx.enter_context(tc.tile_pool(name="sbA", bufs=2))
            psA = ctx.enter_context(tc.tile_pool(name="psA", bufs=2,
                                                 space="PSUM"))

            _xT_tiles = {}

            def emit_phase_a_block(m, n):
                if n == 0:
                    xT = sbA.tile([128, 1024], BF16, tag="xT", name=f"xT_{m}")
                    nc.sync.dma_start(out=xT[:], in_=xT_d[m])
                    _xT_tiles[m] = xT
                xT = _xT_tiles[m]
                pps = psA.tile([128, 512], F32, tag="psA", name=f"psA_{m}_{n}")
                for k in range(8):
                    nc.tensor.matmul(
                        out=pps[:],
                        lhsT=xT[:, k * 128:(k + 1) * 128],
                        rhs=wiT[:, k * G3 + 512 * n: k * G3 + 512 * (n + 1)],
                        start=(k == 0), stop=(k == 7))
                nc.vector.tensor_tensor(
                    out=xp[:, G3 * m + 512 * n: G3 * m + 512 * (n + 1)],
                    in0=pps[:], in1=biasb[:, 512 * n:512 * (n + 1)],
                    op=ALU.add)

            def emit_phase_a(m):
                for n in range(3):
                    emit_phase_a_block(m, n)

            emit_phase_a(0)
            emit_phase_a(1)

            # -------- scan --------
            ps = ctx.enter_context(tc.tile_pool(name="ps", bufs=2, space="PSUM"))
            pso = ctx.enter_context(tc.tile_pool(name="pso", bufs=2, space="PSUM"))
            ps2 = ctx.enter_context(tc.tile_pool(name="ps2", bufs=2, space="PSUM"))

            def emit_sel_mms(t):
                # x_proj selection matmuls for step t: no dependency on h, so
                # they are issued early to fill the PE idle window. The o-gate
                # columns go to a separate PSUM tile: PSUM dependency tracking
                # is bank-coarse, and sharing a bank would serialize the
                # o-phase matmuls behind sig_ij's read.
                m, tl = t >> 4, t & 15
                gps = ps.tile([128, 256], F32, tag="gps", name=f"gps_{t}")
                gpo = pso.tile([128, 128], F32, tag="gpo", name=f"gpo_{t}")
                for g in range(NSTRIP):
                    nc.tensor.matmul(
                        out=gps[32 * g:32 * (g + 1), :],
                        lhsT=sel[:, 32 * tl:32 * tl + 32],
                        rhs=xp[:, G3 * m + SW * g: G3 * m + SW * g + 256],
                        start=True, stop=False, tile_position=(0, 32 * g),
                        skip_group_check=True)
                    nc.tensor.matmul(
                        out=gpo[32 * g:32 * (g + 1), :],
                        lhsT=sel[:, 32 * tl:32 * tl + 32],
                        rhs=xp[:, G3 * m + SW * g + 256: G3 * m + SW * (g + 1)],
                        start=True, stop=False, tile_position=(0, 32 * g),
                        skip_group_check=True)
                return gps, gpo

            gps, gpo = emit_sel_mms(0)
            for t in range(S):
                # one 8-matmul phase-A block per ~5 steps: small enough to
                # hide in each step's PE idle instead of one 5us bubble
                if t % 16 in (0, 5, 10) and (t >> 4) + 2 < NT:
                    emit_phase_a_block((t >> 4) + 2, (0, 5, 10).index(t % 16))
                # recurrent matmuls, round-major so each round only needs its
                # own chunk(s) of hT (pipelines with the chunked copies below)
                def rec_mms(out_t, c0, c1):
                    for k in range(4):
                        for g in range(NSTRIP):
                            nc.tensor.matmul(
                                out=out_t[32 * g:32 * (g + 1), 0:c1 - c0],
                                lhsT=hT[:, 32 * k:32 * (k + 1)],
                                rhs=whT[:, k * G3 + SW * g + c0:
                                         k * G3 + SW * g + c1],
                                start=False, stop=(k == 3),
                                tile_position=(0, 32 * g),
                                skip_group_check=True)
                Sg = sb.tile([128, SW], BF16, tag="Sg", name=f"Sg_{t}")
                rec_mms(gps, 0, 256)   # i, j gate columns: the critical input
                nc.scalar.activation(Sg[:, 0:256], gps[:], AF.Sigmoid)
                rec_mms(gpo, 256, SW)  # o gate columns: consumed later by h
                nc.scalar.activation(Sg[:, 256:SW], gpo[:], AF.Sigmoid)
                d = sb.tile([128, 128], BF16, tag="d", name=f"d_{t}")
                nc.vector.tensor_tensor(out=d[:], in0=Sg[:, 128:256], in1=cp[:],
                                        op=ALU.subtract)
                e = sb.tile([128, 128], BF16, tag="e", name=f"e_{t}")
                nc.vector.tensor_tensor(out=e[:], in0=Sg[:, 0:128], in1=d[:],
                                        op=ALU.mult)
                nc.vector.tensor_tensor(out=cp[:], in0=cp[:], in1=e[:], op=ALU.add)
                tc_t = sb.tile([128, 128], BF16, tag="tc", name=f"tc_{t}")
                nc.scalar.activation(tc_t[:], cp[:], AF.Tanh, bias=neg1[:, 0:1],
                                     scale=2.0)
                if t % 4 == 0:
                    hst = sbh.tile([128, 512], BF16, tag="hst", name=f"hst_{t}")
                h = hst[:, 128 * (t % 4):128 * (t % 4 + 1)]
                nc.vector.tensor_tensor(out=h, in0=tc_t[:], in1=Sg[:, 256:SW],
                                        op=ALU.mult)
                if t % 4 == 3:
                    nc.sync.dma_start(out=out_d[:, :, t - 3:t + 1, :], in_=hst[:])
                if t < S - 1:
                    gps, gpo = emit_sel_mms(t + 1)
                    # chunked transpose: h partition-group -> hT column chunk;
                    # round k of the next step starts after copy k
                    hp = ps2.tile([128, 128], BF16, tag="hp", name=f"hp_{t}")
                    for j in range(2):
                        nc.tensor.transpose(
                            out=hp[:, 64 * j:64 * (j + 1)],
                            in_=hst[64 * j:64 * (j + 1),
                                    128 * (t % 4):128 * (t % 4 + 1)],
                            identity=ident_bf[64 * j:64 * (j + 1),
                                              64 * j:64 * (j + 1)],
                            tile_position=(64 * j, 0))
                        nc.vector.tensor_copy(
                            out=hT[:, 64 * j:64 * (j + 1)],
                            in_=hp[:, 64 * j:64 * (j + 1)])
    return nc


def prep_weights(Wi, bi, Wh, bh, h0, c0):
    perm = []
    for g in range(4):
        perm += list(range(128 * g, 128 * (g + 1)))
        perm += list(range(H + 128 * g, H + 128 * (g + 1)))
        perm += list(range(2 * H + 128 * g, 2 * H + 128 * (g + 1)))
    perm = np.array(perm)
    scale = np.ones(G3, np.float32)
    for g in range(4):
        scale[384 * g + 128:384 * g + 256] = 2.0  # j gates doubled
    Wi_p = (np.asarray(Wi)[perm] * scale[:, None]).astype(np.float32)
    Wh_p = (np.asarray(Wh)[perm] * scale[:, None]).astype(np.float32)
    bias = ((np.asarray(bi) + np.asarray(bh))[perm] * scale).astype(np.float32)
    h0f = np.asarray(h0).reshape(-1).astype(np.float32)
    c0pf = ((1.0 + np.asarray(c0).reshape(-1)) / 2.0).astype(np.float32)
    hT0 = np.zeros((128, 128), np.float32)
    cp0 = np.zeros((128, 128), np.float32)
    for k in range(4):
        hT0[:, 32 * k:32 * k + 8] = h0f[128 * k:128 * (k + 1)][:, None]
        cp0[32 * k:32 * k + 8, :] = c0pf[128 * k:128 * (k + 1)][None, :]
    bf16 = np.float16
    if FP8_REC:
        import ml_dtypes
        # whT[p, k, col] = Wh_p.T[128k + p, col]
        whT = np.ascontiguousarray(
            Wh_p.T.reshape(4, 128, G3).transpose(1, 0, 2)
        ).astype(ml_dtypes.float8_e4m3)
        hT0_o = hT0.reshape(128, 4, 32).astype(ml_dtypes.float8_e4m3)
    else:
        whT = np.ascontiguousarray(Wh_p.T).astype(bf16)
        hT0_o = hT0.astype(bf16)
    return {
        "wiT": np.ascontiguousarray(Wi_p.T).astype(bf16),
        "whT": whT,
        "biasb": np.ascontiguousarray(
            np.broadcast_to(bias, (128, G3))).astype(bf16),
    }, hT0_o, cp0.astype(bf16)


def _sel_matrix():
    sel = np.zeros((128, 16 * 32), np.float16)
    for tl in range(16):
        for b in range(B_CORE):
            sel[8 * tl + b, 32 * tl + b] = 1.0
    return sel


def _to_xT(x_core):
    """[8, 512, 1024] -> [NT, 128, 8, 128]: [m, f, k, 8*tl+b]."""
    a = x_core.reshape(B_CORE, NT, 16, 8, 128)  # b, m, tl, k, f
    return np.ascontiguousarray(a.transpose(1, 4, 3, 2, 0)).reshape(NT, 128, 8, 128)


_NC_CACHE = {}


def _get_nc():
    if "nc" not in _NC_CACHE:
        _NC_CACHE["nc"] = build_nc()
    return _NC_CACHE["nc"]


def make_in_maps(input, sent_len, fwd_h0, fwd_c0, fwd_Wi, fwd_bi, fwd_Wh, fwd_bh,
                 bwd_h0, bwd_c0, bwd_Wi, bwd_bi, bwd_Wh, bwd_bh, **_unused):
    x = np.asarray(input, np.float32).astype(np.float16)
    sent = np.asarray(sent_len).astype(np.int64)
    # per-batch reversal index: first sent_len[b] steps reversed, rest kept
    tt = np.arange(S)[None, :]
    L = sent[:, None]
    rev_idx = np.where(tt < L, L - 1 - tt, tt)  # [B, S]
    fwd_w, fwd_hT0, fwd_cp0 = prep_weights(fwd_Wi, fwd_bi, fwd_Wh, fwd_bh,
                                           fwd_h0, fwd_c0)
    bwd_w, bwd_hT0, bwd_cp0 = prep_weights(bwd_Wi, bwd_bi, bwd_Wh, bwd_bh,
                                           bwd_h0, bwd_c0)
    sel = _sel_matrix()
    in_maps = []
    for c in range(8):
        bs = (c % 4) * B_CORE
        xc = x[bs:bs + B_CORE]
        if c >= 4:
            xc = np.take_along_axis(xc, rev_idx[bs:bs + B_CORE, :, None], axis=1)
        w, hT0, cp0 = (fwd_w, fwd_hT0, fwd_cp0) if c < 4 else \
                      (bwd_w, bwd_hT0, bwd_cp0)
        m = dict(w)
        m["xT"] = _to_xT(xc)
        m["sel"] = sel
        m["hT0"] = hT0
        m["cp0"] = cp0
        in_maps.append(m)
    return in_maps, rev_idx


def kernel(**inputs):
    from concourse.bass_utils import run_bass_kernel_spmd

    in_maps, rev_idx = make_in_maps(**inputs)
    nc = _get_nc()
    res = run_bass_kernel_spmd(nc, in_maps, list(range(8)))
    out = np.zeros((B, S, 2 * H), np.float32)
    for c in range(8):
        bs = (c % 4) * B_CORE
        o = res.results[c]["out"]  # [4, 32, S, 128]
        o = o[:, :B_CORE].transpose(1, 2, 0, 3).reshape(B_CORE, S, H)
        if c < 4:
            out[bs:bs + B_CORE, :, :H] = o
        else:
            idx = rev_idx[bs:bs + B_CORE]
            out[bs:bs + B_CORE, :, H:] = np.take_along_axis(
                o, idx[:, :, None], axis=1)
    return out
